# revision 10
# baseline (speedup 1.0000x reference)
"""MultiHeadAttention Trainium2 Bass kernel (8 NeuronCores, SPMD, no collectives).

Problem (hardcoded): B=4, S=1024, D=1024, H=16, DH=64, fp32 I/O.
  q = split_heads(queries @ Wq) * DH**-0.5
  k = split_heads(keys @ Wk); v = split_heads(values @ Wv)
  logits = q k^T, masked (key mask -> -1e18)
  attention_weights = logits.sum(heads)/H      [B,S,S]
  outputs = (softmax(logits) v) @ Wo           [B,S,D]

Sharding: core c -> (batch b = c//2, head-half g = c%2).  Each core computes
8 heads' worth of projections/attention for one batch and a PARTIAL
  - outputs contribution  (its heads' rows of Wo)        [S,D] f32
  - attention-weight head-sum, stored transposed [ks,qs] [S,S] f32
Host sums the two partials per batch (and transposes attw).  No device
collectives; host-side shard/unshard is numpy.

Device layouts are feature-major ("transposed"): host passes queries^T etc.
All matmul inputs bf16, PSUM accumulation f32.  Softmax without max-
subtraction (logits are O(10); exp stays in fp32 range), denominator via an
appended ones-column on V (M=65 matmuls), mask applied as an additive
per-partition bias (-1e18) inside the Exp activation.
"""

import os

import numpy as np
import ml_dtypes

B, S, D, H = 4, 1024, 1024, 16
DH = D // H
SCALE = DH ** -0.5
NEG = -1e18
P = 128
HLOC = H // 2          # heads per core
DHALF = D // 2         # projection cols per core
ET = D // P            # 8 contraction tiles over D
DC = DHALF // P        # 4 chunks of local head dims
KC = S // P            # 8 key chunks
NW = 512               # matmul moving width
NQ2 = S // NW          # 2 query halves
bf16 = ml_dtypes.bfloat16

_CACHE: dict = {}


def _emit(tc, nc, aps):
    import concourse.bass as bass
    from concourse import mybir

    f32 = mybir.dt.float32
    bf = mybir.dt.bfloat16
    xq, xk, xv, wq, wk, wv, wo, mb, outp, attwT = aps

    from contextlib import ExitStack

    es = ExitStack()
    consts = es.enter_context(tc.tile_pool(name="consts", bufs=1))
    xpool = es.enter_context(tc.tile_pool(name="xpool", bufs=3))
    wpool = es.enter_context(tc.tile_pool(name="wpool", bufs=3))
    wopool = es.enter_context(tc.tile_pool(name="wopool", bufs=1))
    persist = es.enter_context(tc.tile_pool(name="persist", bufs=1))
    epool = es.enter_context(tc.tile_pool(name="epool", bufs=1))
    stage = es.enter_context(tc.tile_pool(name="stage", bufs=2))
    small = es.enter_context(tc.tile_pool(name="small", bufs=1))
    # PSUM budget (8 banks): l0/l1 [128,1024] = 4 banks, c0/c1 [128,1024] = 4.
    psL = es.enter_context(tc.tile_pool(name="psL", bufs=1, space="PSUM"))
    psC = es.enter_context(tc.tile_pool(name="psC", bufs=1, space="PSUM"))

    _lc = [0]
    _ln = [0]

    def ltile(cols):
        _lc[0] ^= 1
        _ln[0] += 1
        return psL.tile([P, cols], f32, tag=f"l{_lc[0]}", name=f"lt{_ln[0]}")

    with es:
        mb_sb = consts.tile([P, KC], f32, tag="mb")
        nc.sync.dma_start(out=mb_sb[:], in_=mb[:])
        mbh_sb = consts.tile([P, KC], f32, tag="mbh")
        nc.vector.tensor_scalar_mul(out=mbh_sb[:], in0=mb_sb[:], scalar1=0.5)

        # ---- inputs (bf16, feature-major).  Weights first (small), then acts.
        wq_sb = wpool.tile([P, ET, DHALF], bf, tag="w", name="wq")
        nc.sync.dma_start(out=wq_sb[:], in_=wq.rearrange("(t p) m -> p t m", p=P))
        wk_sb = wpool.tile([P, ET, DHALF], bf, tag="w", name="wk")
        nc.sync.dma_start(out=wk_sb[:], in_=wk.rearrange("(t p) m -> p t m", p=P))
        xq_sb = xpool.tile([P, ET, S], bf, tag="x", name="xq")
        nc.sync.dma_start(out=xq_sb[:], in_=xq.rearrange("(t p) s -> p t s", p=P))
        xk_sb = xpool.tile([P, ET, S], bf, tag="x", name="xk")
        nc.sync.dma_start(out=xk_sb[:], in_=xk.rearrange("(t p) s -> p t s", p=P))

        QTc = [persist.tile([P, S], bf, tag=f"QT{i}", name=f"QT{i}") for i in range(DC)]
        KTc = [persist.tile([P, S], bf, tag=f"KT{i}", name=f"KT{i}") for i in range(DC)]
        Vc = [
            persist.tile([P, HLOC * 65], bf, tag=f"V{k}", name=f"V{k}")
            for k in range(KC)
        ]
        ctxc = [
            persist.tile([P, S], bf, tag=f"CX{i}", name=f"CX{i}") for i in range(DC)
        ]

        def proj_chunk(dst, w_sb, x_sb, i):
            # one 128-col chunk of a q/k projection, as two half-groups so the
            # PSUM slot is held only ~2us at a time
            for n in range(NQ2):
                ps = ltile(NW)
                for t in range(ET):
                    nc.tensor.matmul(
                        ps[:],
                        lhsT=w_sb[:, t, i * P:(i + 1) * P],
                        rhs=x_sb[:, t, n * NW:(n + 1) * NW],
                        start=(t == 0),
                        stop=(t == ET - 1),
                    )
                nc.vector.tensor_copy(out=dst[:, n * NW:(n + 1) * NW], in_=ps[:])

        proj_chunk(QTc[0], wq_sb, xq_sb, 0)
        proj_chunk(KTc[0], wk_sb, xk_sb, 0)

        # ---- V projection, per key-chunk tiles (natural [ks, dh] layout with
        # a ones column appended per head for the softmax denominator)
        xv_sb = xpool.tile([P, ET, S], bf, tag="x", name="xv")
        nc.sync.dma_start(out=xv_sb[:], in_=xv.rearrange("(t p) s -> p t s", p=P))
        wv_sb = wpool.tile([P, ET, DHALF], bf, tag="w", name="wv")
        nc.sync.dma_start(out=wv_sb[:], in_=wv.rearrange("(t p) m -> p t m", p=P))
        wo_sb = wopool.tile([P, DC, D], bf, tag="wo")
        nc.sync.dma_start(out=wo_sb[:], in_=wo.rearrange("(i p) n -> p i n", p=P))
        for ck in range(KC):
            ps = ltile(NW)
            for t in range(ET):
                nc.tensor.matmul(
                    ps[:],
                    lhsT=xv_sb[:, t, ck * P:(ck + 1) * P],
                    rhs=wv_sb[:, t, :],
                    start=(t == 0),
                    stop=(t == ET - 1),
                )
            dst = Vc[ck][:].rearrange("p (h c) -> p h c", c=65)
            nc.vector.tensor_copy(
                out=dst[:, :, 0:64], in_=ps[:].rearrange("p (h c) -> p h c", c=64)
            )
            nc.vector.memset(dst[:, :, 64:65], 1.0)

        # ---- head pairs: logits -> exp -> ctx, with the next q/k projection
        # chunk emitted after each pair as PE filler under the ACT-bound phase
        for hp in range(DC):
            pscs = [
                psC.tile([P, S], f32, tag=f"c{s}", name=f"c{s}_{hp}")
                for s in range(2)
            ]
            for ck in range(KC):
                e_tiles = [
                    epool.tile([P, S], bf, tag=f"e{s}_{ck}", name=f"e{s}_{ck}_{hp}")
                    for s in range(2)
                ]
                for s in range(2):
                    po = 64 * s
                    psl = ltile(S)
                    for n in range(NQ2):
                        nc.tensor.matmul(
                            psl[:, n * NW:(n + 1) * NW],
                            lhsT=KTc[hp][po:po + 64, ck * P:(ck + 1) * P],
                            rhs=QTc[hp][po:po + 64, n * NW:(n + 1) * NW],
                            start=True,
                            stop=True,
                        )
                    nc.scalar.activation(
                        out=e_tiles[s][:],
                        in_=psl[:],
                        func=mybir.ActivationFunctionType.Exp,
                        bias=mb_sb[:, ck:ck + 1],
                        scale=1.0,
                    )
                for s in range(2):
                    h = 2 * hp + s
                    for n in range(NQ2):
                        nc.tensor.matmul(
                            pscs[s][0:65, n * NW:(n + 1) * NW],
                            lhsT=Vc[ck][:, h * 65:(h + 1) * 65],
                            rhs=e_tiles[s][:, n * NW:(n + 1) * NW],
                            start=(ck == 0),
                            stop=(ck == KC - 1),
                        )
            # Drain ctx PSUM to SBUF promptly (releases the PSUM slot for the
            # next pair); normalization then runs off the critical path.
            den = small.tile([1, 2 * S], f32, tag="den", name=f"den{hp}")
            cus = []
            for s in range(2):
                cu = small.tile([64, S], bf, tag=f"cu{s}", name=f"cu{s}_{hp}")
                nc.vector.tensor_copy(out=cu[:], in_=pscs[s][0:64, :])
                nc.vector.tensor_copy(
                    out=den[0:1, s * S:(s + 1) * S], in_=pscs[s][64:65, :]
                )
                cus.append(cu)
            nc.vector.reciprocal(out=den[:], in_=den[:])
            for s in range(2):
                rb = small.tile([64, S], f32, tag=f"rb{s}", name=f"rb{s}_{hp}")
                nc.gpsimd.partition_broadcast(rb[:], den[0:1, s * S:(s + 1) * S])
                nc.vector.tensor_mul(
                    out=ctxc[hp][64 * s:64 * s + 64, :],
                    in0=cus[s][:],
                    in1=rb[:],
                )
            if hp + 1 < DC:
                proj_chunk(QTc[hp + 1], wq_sb, xq_sb, hp + 1)
                proj_chunk(KTc[hp + 1], wk_sb, xk_sb, hp + 1)

        # ---- attention-weight head-sum: attwT_partial = (KT^T QT)/H + mask/2
        for ck in range(KC):
            ps = ltile(S)
            for n in range(NQ2):
                for i in range(DC):
                    nc.tensor.matmul(
                        ps[:, n * NW:(n + 1) * NW],
                        lhsT=KTc[i][:, ck * P:(ck + 1) * P],
                        rhs=QTc[i][:, n * NW:(n + 1) * NW],
                        start=(i == 0),
                        stop=(i == DC - 1),
                    )
            st = stage.tile([P, S], f32, tag="attw_st", name=f"awst{ck}")
            nc.vector.tensor_scalar(
                out=st[:],
                in0=ps[:],
                scalar1=1.0 / H,
                scalar2=mbh_sb[:, ck:ck + 1],
                op0=mybir.AluOpType.mult,
                op1=mybir.AluOpType.add,
            )
            nc.sync.dma_start(out=attwT[ck * P:(ck + 1) * P, :], in_=st[:])

        # ---- output projection: outp_partial = ctxT^T @ Wo_rows
        for m in range(KC):
            ps = ltile(S)
            for n in range(NQ2):
                for i in range(DC):
                    nc.tensor.matmul(
                        ps[:, n * NW:(n + 1) * NW],
                        lhsT=ctxc[i][:, m * P:(m + 1) * P],
                        rhs=wo_sb[:, i, n * NW:(n + 1) * NW],
                        start=(i == 0),
                        stop=(i == DC - 1),
                    )
            st = stage.tile([P, S], f32, tag="out_st", name=f"ost{m}")
            nc.vector.tensor_copy(out=st[:], in_=ps[:])
            nc.sync.dma_start(out=outp[m * P:(m + 1) * P, :], in_=st[:])


def build():
    if "nc" in _CACHE:
        return _CACHE["nc"]
    import concourse.tile as tile
    from concourse import bacc, mybir

    f32 = mybir.dt.float32
    bf = mybir.dt.bfloat16
    nc = bacc.Bacc("TRN2", target_bir_lowering=False, debug=False)
    xq = nc.dram_tensor("xq", [D, S], bf, kind="ExternalInput").ap()
    xk = nc.dram_tensor("xk", [D, S], bf, kind="ExternalInput").ap()
    xv = nc.dram_tensor("xv", [D, S], bf, kind="ExternalInput").ap()
    wq = nc.dram_tensor("wq", [D, DHALF], bf, kind="ExternalInput").ap()
    wk = nc.dram_tensor("wk", [D, DHALF], bf, kind="ExternalInput").ap()
    wv = nc.dram_tensor("wv", [D, DHALF], bf, kind="ExternalInput").ap()
    wo = nc.dram_tensor("wo", [DHALF, D], bf, kind="ExternalInput").ap()
    mb = nc.dram_tensor("mb", [P, KC], f32, kind="ExternalInput").ap()
    outp = nc.dram_tensor("outp", [S, D], f32, kind="ExternalOutput").ap()
    attwT = nc.dram_tensor("attwT", [S, S], f32, kind="ExternalOutput").ap()

    with tile.TileContext(nc) as tc:
        _emit(tc, nc, (xq, xk, xv, wq, wk, wv, wo, mb, outp, attwT))
    nc.compile()
    _CACHE["nc"] = nc
    return nc


def make_in_maps(queries, keys, values, Wq, Wk, Wv, Wo, mask):
    queries = np.asarray(queries, np.float32)
    keys = np.asarray(keys, np.float32)
    values = np.asarray(values, np.float32)
    Wqs = np.asarray(Wq, np.float32) * SCALE
    Wk = np.asarray(Wk, np.float32)
    Wv = np.asarray(Wv, np.float32)
    Wo = np.asarray(Wo, np.float32)
    mask = np.asarray(mask)

    xT = {}
    for b in range(B):
        xT[b] = tuple(
            np.ascontiguousarray(a[b].T).astype(bf16)
            for a in (queries, keys, values)
        )
    in_maps = []
    for c in range(8):
        b, g = divmod(c, 2)
        sl = slice(g * DHALF, (g + 1) * DHALF)
        mbias = np.where(mask[b, 0], np.float32(NEG), np.float32(0.0)).astype(
            np.float32
        )
        in_maps.append(
            {
                "xq": xT[b][0],
                "xk": xT[b][1],
                "xv": xT[b][2],
                "wq": np.ascontiguousarray(Wqs[:, sl]).astype(bf16),
                "wk": np.ascontiguousarray(Wk[:, sl]).astype(bf16),
                "wv": np.ascontiguousarray(Wv[:, sl]).astype(bf16),
                "wo": np.ascontiguousarray(Wo[sl, :]).astype(bf16),
                "mb": np.ascontiguousarray(mbias.reshape(KC, P).T),
            }
        )
    return in_maps


def gather(results):
    outputs = np.empty((B, S, D), np.float32)
    attw = np.empty((B, S, S), np.float32)
    for b in range(B):
        outputs[b] = results[2 * b]["outp"] + results[2 * b + 1]["outp"]
        attw[b] = (results[2 * b]["attwT"] + results[2 * b + 1]["attwT"]).T
    return outputs, attw


def _enable_ntff_tracing():
    """Bridge trn_agent_boot's ctypes NTFF hook into antenv.axon_hooks
    (absent on this image), so run_bass_kernel_spmd(trace=True) can pull
    NTFF profiles back from the terminal. Also neuter the S3 artifact
    upload, which has no credentials here."""
    import sys
    import types

    try:
        from antenv.axon_hooks import get_axon_ntff_profile_hook  # noqa: F401
    except ImportError:
        import antenv
        from trn_agent_boot.trn_boot import _ntff_profile_via_ctypes

        hook = _ntff_profile_via_ctypes("/opt/axon/libaxon_pjrt.so")
        mod = types.ModuleType("antenv.axon_hooks")
        mod.get_axon_ntff_profile_hook = lambda: hook
        mod.set_axon_ntff_profile_hook = lambda h: None
        sys.modules["antenv.axon_hooks"] = mod
        antenv.axon_hooks = mod

    import concourse.bass_utils as bu

    bu.upload_artifacts = lambda tmpdir: tmpdir


def run(inputs, trace=False):
    from concourse.bass_utils import run_bass_kernel_spmd

    if trace:
        _enable_ntff_tracing()
    nc = build()
    in_maps = make_in_maps(**inputs)
    res = run_bass_kernel_spmd(nc, in_maps, list(range(8)), trace=trace)
    outputs, attw = gather(res.results)
    return (outputs, attw), res


def kernel(queries, keys, values, Wq, Wk, Wv, Wo, mask):
    trace = bool(os.environ.get("KERNEL_TRACE"))
    (outputs, attw), res = run(
        dict(
            queries=queries,
            keys=keys,
            values=values,
            Wq=Wq,
            Wk=Wk,
            Wv=Wv,
            Wo=Wo,
            mask=mask,
        ),
        trace=trace,
    )
    if trace and res.exec_time_ns is not None:
        print(f"HW exec time: {res.exec_time_ns} ns")
    return outputs, attw


# revision 15
# speedup vs baseline: 1.2066x; 1.2066x over previous
"""MultiHeadAttention Trainium2 Bass kernel (8 NeuronCores, SPMD, no collectives).

Problem (hardcoded): B=4, S=1024, D=1024, H=16, DH=64, fp32 I/O.
  q = split_heads(queries @ Wq) * DH**-0.5
  k = split_heads(keys @ Wk); v = split_heads(values @ Wv)
  logits = q k^T, masked (key mask -> -1e18)
  attention_weights = logits.sum(heads)/H      [B,S,S]
  outputs = (softmax(logits) v) @ Wo           [B,S,D]

Sharding: core c -> (batch b = c//2, head-half g = c%2).  Each core computes
8 heads' worth of projections/attention for one batch and a PARTIAL
  - outputs contribution  (its heads' rows of Wo)        [S,D] f32
  - attention-weight head-sum, stored transposed [ks,qs] [S,S] f32
Host sums the two partials per batch (and transposes attw).  No device
collectives; host-side shard/unshard is numpy.

Device layouts are feature-major ("transposed"): host passes queries^T etc.
All matmul inputs bf16, PSUM accumulation f32.  Softmax without max-
subtraction (logits are O(10); exp stays in fp32 range), denominator via an
appended ones-column on V (M=65 matmuls), mask applied as an additive
per-partition bias (-1e18) inside the Exp activation.
"""

import os

import numpy as np
import ml_dtypes

B, S, D, H = 4, 1024, 1024, 16
DH = D // H
SCALE = DH ** -0.5
NEG = -1e18
P = 128
HLOC = H // 2          # heads per core
DHALF = D // 2         # projection cols per core
ET = D // P            # 8 contraction tiles over D
DC = DHALF // P        # 4 chunks of local head dims
KC = S // P            # 8 key chunks
NW = 512               # matmul moving width
NQ2 = S // NW          # 2 query halves
bf16 = ml_dtypes.bfloat16

_CACHE: dict = {}


def _emit(tc, nc, aps):
    import concourse.bass as bass
    from concourse import mybir

    f32 = mybir.dt.float32
    bf = mybir.dt.bfloat16
    xq, xk, xv, wq, wk, wv, wo, mb, outp, attwT = aps

    from contextlib import ExitStack

    es = ExitStack()
    consts = es.enter_context(tc.tile_pool(name="consts", bufs=1))
    xpool = es.enter_context(tc.tile_pool(name="xpool", bufs=1))
    wpool = es.enter_context(tc.tile_pool(name="wpool", bufs=1))
    wopool = es.enter_context(tc.tile_pool(name="wopool", bufs=1))
    persist = es.enter_context(tc.tile_pool(name="persist", bufs=1))
    epool = es.enter_context(tc.tile_pool(name="epool", bufs=1))
    stage = es.enter_context(tc.tile_pool(name="stage", bufs=2))
    small = es.enter_context(tc.tile_pool(name="small", bufs=1))
    # PSUM budget (8 banks): l0/l1 [128,1024] = 4 banks, c0/c1 [128,1024] = 4.
    psL = es.enter_context(tc.tile_pool(name="psL", bufs=1, space="PSUM"))
    psC = es.enter_context(tc.tile_pool(name="psC", bufs=1, space="PSUM"))

    _lc = [0]
    _ln = [0]

    def ltile(cols):
        _lc[0] ^= 1
        _ln[0] += 1
        return psL.tile([P, cols], f32, tag=f"l{_lc[0]}", name=f"lt{_ln[0]}")

    with es:
        mb_sb = consts.tile([P, KC], f32, tag="mb")
        nc.sync.dma_start(out=mb_sb[:], in_=mb[:])
        mbh_sb = consts.tile([P, KC], f32, tag="mbh")
        nc.vector.tensor_scalar_mul(out=mbh_sb[:], in0=mb_sb[:], scalar1=0.5)

        # ---- inputs (bf16, feature-major), split per contraction tile so the
        # first projection matmuls start as soon as the first slices land.
        def load_split(pool, dram, name, width):
            dv = dram.rearrange("(t p) m -> t p m", p=P)
            tiles = []
            for t in range(ET):
                tl = pool.tile([P, width], bf, tag=f"{name}{t}", name=f"{name}{t}")
                nc.sync.dma_start(out=tl[:], in_=dv[t])
                tiles.append(tl)
            return tiles

        wq_sb = load_split(wpool, wq, "wq", DHALF)
        xq_sb = load_split(xpool, xq, "xq", S)
        wk_sb = load_split(wpool, wk, "wk", DHALF)
        xk_sb = load_split(xpool, xk, "xk", S)

        QTc = [persist.tile([P, S], bf, tag=f"QT{i}", name=f"QT{i}") for i in range(DC)]
        KTc = [persist.tile([P, S], bf, tag=f"KT{i}", name=f"KT{i}") for i in range(DC)]
        Vc = [
            persist.tile([P, HLOC * 65], bf, tag=f"V{k}", name=f"V{k}")
            for k in range(KC)
        ]
        ctxc = [
            persist.tile([P, S], bf, tag=f"CX{i}", name=f"CX{i}") for i in range(DC)
        ]

        def proj_chunk(dst, w_sb, x_sb, i):
            # one 128-col chunk of a q/k projection, as two half-groups so the
            # PSUM slot is held only ~2us at a time
            for n in range(NQ2):
                ps = ltile(NW)
                for t in range(ET):
                    nc.tensor.matmul(
                        ps[:],
                        lhsT=w_sb[t][:, i * P:(i + 1) * P],
                        rhs=x_sb[t][:, n * NW:(n + 1) * NW],
                        start=(t == 0),
                        stop=(t == ET - 1),
                    )
                nc.vector.tensor_copy(out=dst[:, n * NW:(n + 1) * NW], in_=ps[:])

        proj_chunk(QTc[0], wq_sb, xq_sb, 0)
        proj_chunk(KTc[0], wk_sb, xk_sb, 0)

        # ---- V projection, per key-chunk tiles (natural [ks, dh] layout with
        # a ones column appended per head for the softmax denominator)
        wv_sb = load_split(wpool, wv, "wv", DHALF)
        xv_sb = load_split(xpool, xv, "xv", S)
        wo_sb = wopool.tile([P, DC, D], bf, tag="wo")
        nc.sync.dma_start(out=wo_sb[:], in_=wo.rearrange("(i p) n -> p i n", p=P))
        for ck in range(KC):
            ps = ltile(NW)
            for t in range(ET):
                nc.tensor.matmul(
                    ps[:],
                    lhsT=xv_sb[t][:, ck * P:(ck + 1) * P],
                    rhs=wv_sb[t][:],
                    start=(t == 0),
                    stop=(t == ET - 1),
                )
            dst = Vc[ck][:].rearrange("p (h c) -> p h c", c=65)
            nc.vector.tensor_copy(
                out=dst[:, :, 0:64], in_=ps[:].rearrange("p (h c) -> p h c", c=64)
            )
            nc.vector.memset(dst[:, :, 64:65], 1.0)

        # ---- head pairs: logits -> exp -> ctx, with the next q/k projection
        # chunk emitted after each pair as PE filler under the ACT-bound phase
        for hp in range(DC):
            pscs = [
                psC.tile([P, S], f32, tag=f"c{s}", name=f"c{s}_{hp}")
                for s in range(2)
            ]
            for ck in range(KC):
                e_tiles = [
                    epool.tile([P, S], bf, tag=f"e{s}_{ck}", name=f"e{s}_{ck}_{hp}")
                    for s in range(2)
                ]
                for s in range(2):
                    po = 64 * s
                    psl = ltile(S)
                    for n in range(NQ2):
                        nc.tensor.matmul(
                            psl[:, n * NW:(n + 1) * NW],
                            lhsT=KTc[hp][po:po + 64, ck * P:(ck + 1) * P],
                            rhs=QTc[hp][po:po + 64, n * NW:(n + 1) * NW],
                            start=True,
                            stop=True,
                        )
                    nc.scalar.activation(
                        out=e_tiles[s][:],
                        in_=psl[:],
                        func=mybir.ActivationFunctionType.Exp,
                        bias=mb_sb[:, ck:ck + 1],
                        scale=1.0,
                    )
                for s in range(2):
                    h = 2 * hp + s
                    for n in range(NQ2):
                        nc.tensor.matmul(
                            pscs[s][0:65, n * NW:(n + 1) * NW],
                            lhsT=Vc[ck][:, h * 65:(h + 1) * 65],
                            rhs=e_tiles[s][:, n * NW:(n + 1) * NW],
                            start=(ck == 0),
                            stop=(ck == KC - 1),
                        )
            # Drain ctx PSUM to SBUF promptly (releases the PSUM slot for the
            # next pair); normalization then runs off the critical path.
            den = small.tile([1, 2 * S], f32, tag="den", name=f"den{hp}")
            cus = []
            for s in range(2):
                cu = small.tile([64, S], bf, tag=f"cu{s}", name=f"cu{s}_{hp}")
                nc.vector.tensor_copy(out=cu[:], in_=pscs[s][0:64, :])
                nc.vector.tensor_copy(
                    out=den[0:1, s * S:(s + 1) * S], in_=pscs[s][64:65, :]
                )
                cus.append(cu)
            nc.vector.reciprocal_approx_fast(out=den[:], in_=den[:])
            for s in range(2):
                rb = small.tile([64, S], f32, tag=f"rb{s}", name=f"rb{s}_{hp}")
                nc.gpsimd.partition_broadcast(rb[:], den[0:1, s * S:(s + 1) * S])
                nc.vector.tensor_mul(
                    out=ctxc[hp][64 * s:64 * s + 64, :],
                    in0=cus[s][:],
                    in1=rb[:],
                )
            if hp + 1 < DC:
                proj_chunk(QTc[hp + 1], wq_sb, xq_sb, hp + 1)
                proj_chunk(KTc[hp + 1], wk_sb, xk_sb, hp + 1)

        # ---- attention-weight head-sum: attwT_partial = (KT^T QT)/H + mask/2
        for ck in range(KC):
            ps = ltile(S)
            for n in range(NQ2):
                for i in range(DC):
                    nc.tensor.matmul(
                        ps[:, n * NW:(n + 1) * NW],
                        lhsT=KTc[i][:, ck * P:(ck + 1) * P],
                        rhs=QTc[i][:, n * NW:(n + 1) * NW],
                        start=(i == 0),
                        stop=(i == DC - 1),
                    )
            st = stage.tile([P, S], f32, tag="attw_st", name=f"awst{ck}")
            nc.vector.tensor_scalar(
                out=st[:],
                in0=ps[:],
                scalar1=1.0 / H,
                scalar2=mbh_sb[:, ck:ck + 1],
                op0=mybir.AluOpType.mult,
                op1=mybir.AluOpType.add,
            )
            nc.sync.dma_start(out=attwT[ck * P:(ck + 1) * P, :], in_=st[:])

        # ---- output projection: outp_partial = ctxT^T @ Wo_rows
        for m in range(KC):
            ps = ltile(S)
            for n in range(NQ2):
                for i in range(DC):
                    nc.tensor.matmul(
                        ps[:, n * NW:(n + 1) * NW],
                        lhsT=ctxc[i][:, m * P:(m + 1) * P],
                        rhs=wo_sb[:, i, n * NW:(n + 1) * NW],
                        start=(i == 0),
                        stop=(i == DC - 1),
                    )
            st = stage.tile([P, S], f32, tag="out_st", name=f"ost{m}")
            nc.vector.tensor_copy(out=st[:], in_=ps[:])
            nc.sync.dma_start(out=outp[m * P:(m + 1) * P, :], in_=st[:])


def build():
    if "nc" in _CACHE:
        return _CACHE["nc"]
    import concourse.tile as tile
    from concourse import bacc, mybir

    f32 = mybir.dt.float32
    bf = mybir.dt.bfloat16
    nc = bacc.Bacc("TRN2", target_bir_lowering=False, debug=False)
    xq = nc.dram_tensor("xq", [D, S], bf, kind="ExternalInput").ap()
    xk = nc.dram_tensor("xk", [D, S], bf, kind="ExternalInput").ap()
    xv = nc.dram_tensor("xv", [D, S], bf, kind="ExternalInput").ap()
    wq = nc.dram_tensor("wq", [D, DHALF], bf, kind="ExternalInput").ap()
    wk = nc.dram_tensor("wk", [D, DHALF], bf, kind="ExternalInput").ap()
    wv = nc.dram_tensor("wv", [D, DHALF], bf, kind="ExternalInput").ap()
    wo = nc.dram_tensor("wo", [DHALF, D], bf, kind="ExternalInput").ap()
    mb = nc.dram_tensor("mb", [P, KC], f32, kind="ExternalInput").ap()
    outp = nc.dram_tensor("outp", [S, D], f32, kind="ExternalOutput").ap()
    attwT = nc.dram_tensor("attwT", [S, S], f32, kind="ExternalOutput").ap()

    with tile.TileContext(nc) as tc:
        _emit(tc, nc, (xq, xk, xv, wq, wk, wv, wo, mb, outp, attwT))
    nc.compile()
    _CACHE["nc"] = nc
    return nc


def make_in_maps(queries, keys, values, Wq, Wk, Wv, Wo, mask):
    queries = np.asarray(queries, np.float32)
    keys = np.asarray(keys, np.float32)
    values = np.asarray(values, np.float32)
    Wqs = np.asarray(Wq, np.float32) * SCALE
    Wk = np.asarray(Wk, np.float32)
    Wv = np.asarray(Wv, np.float32)
    Wo = np.asarray(Wo, np.float32)
    mask = np.asarray(mask)

    xT = {}
    for b in range(B):
        xT[b] = tuple(
            np.ascontiguousarray(a[b].T).astype(bf16)
            for a in (queries, keys, values)
        )
    in_maps = []
    for c in range(8):
        b, g = divmod(c, 2)
        sl = slice(g * DHALF, (g + 1) * DHALF)
        mbias = np.where(mask[b, 0], np.float32(NEG), np.float32(0.0)).astype(
            np.float32
        )
        in_maps.append(
            {
                "xq": xT[b][0],
                "xk": xT[b][1],
                "xv": xT[b][2],
                "wq": np.ascontiguousarray(Wqs[:, sl]).astype(bf16),
                "wk": np.ascontiguousarray(Wk[:, sl]).astype(bf16),
                "wv": np.ascontiguousarray(Wv[:, sl]).astype(bf16),
                "wo": np.ascontiguousarray(Wo[sl, :]).astype(bf16),
                "mb": np.ascontiguousarray(mbias.reshape(KC, P).T),
            }
        )
    return in_maps


def gather(results):
    outputs = np.empty((B, S, D), np.float32)
    attw = np.empty((B, S, S), np.float32)
    for b in range(B):
        outputs[b] = results[2 * b]["outp"] + results[2 * b + 1]["outp"]
        attw[b] = (results[2 * b]["attwT"] + results[2 * b + 1]["attwT"]).T
    return outputs, attw


def _enable_ntff_tracing():
    """Bridge trn_agent_boot's ctypes NTFF hook into antenv.axon_hooks
    (absent on this image), so run_bass_kernel_spmd(trace=True) can pull
    NTFF profiles back from the terminal. Also neuter the S3 artifact
    upload, which has no credentials here."""
    import sys
    import types

    try:
        from antenv.axon_hooks import get_axon_ntff_profile_hook  # noqa: F401
    except ImportError:
        import antenv
        from trn_agent_boot.trn_boot import _ntff_profile_via_ctypes

        hook = _ntff_profile_via_ctypes("/opt/axon/libaxon_pjrt.so")
        mod = types.ModuleType("antenv.axon_hooks")
        mod.get_axon_ntff_profile_hook = lambda: hook
        mod.set_axon_ntff_profile_hook = lambda h: None
        sys.modules["antenv.axon_hooks"] = mod
        antenv.axon_hooks = mod

    import concourse.bass_utils as bu

    bu.upload_artifacts = lambda tmpdir: tmpdir


def run(inputs, trace=False):
    from concourse.bass_utils import run_bass_kernel_spmd

    if trace:
        _enable_ntff_tracing()
    nc = build()
    in_maps = make_in_maps(**inputs)
    res = run_bass_kernel_spmd(nc, in_maps, list(range(8)), trace=trace)
    outputs, attw = gather(res.results)
    return (outputs, attw), res


def kernel(queries, keys, values, Wq, Wk, Wv, Wo, mask):
    trace = bool(os.environ.get("KERNEL_TRACE"))
    (outputs, attw), res = run(
        dict(
            queries=queries,
            keys=keys,
            values=values,
            Wq=Wq,
            Wk=Wk,
            Wv=Wv,
            Wo=Wo,
            mask=mask,
        ),
        trace=trace,
    )
    if trace and res.exec_time_ns is not None:
        print(f"HW exec time: {res.exec_time_ns} ns")
    return outputs, attw


# revision 18
# speedup vs baseline: 1.3297x; 1.1020x over previous
"""MultiHeadAttention Trainium2 Bass kernel (8 NeuronCores, SPMD, no collectives).

Problem (hardcoded): B=4, S=1024, D=1024, H=16, DH=64, fp32 I/O.
  q = split_heads(queries @ Wq) * DH**-0.5
  k = split_heads(keys @ Wk); v = split_heads(values @ Wv)
  logits = q k^T, masked (key mask -> -1e18)
  attention_weights = logits.sum(heads)/H      [B,S,S]
  outputs = (softmax(logits) v) @ Wo           [B,S,D]

Sharding: core c -> (batch b = c//2, head-half g = c%2).  Each core computes
8 heads' worth of projections/attention for one batch and a PARTIAL
  - outputs contribution  (its heads' rows of Wo)        [S,D] f32
  - attention-weight head-sum, stored transposed [ks,qs] [S,S] f32
Host sums the two partials per batch (and transposes attw).  No device
collectives; host-side shard/unshard is numpy.

Device layouts are feature-major ("transposed"): host passes queries^T etc.
All matmul inputs bf16, PSUM accumulation f32.  Softmax without max-
subtraction (logits are O(10); exp stays in fp32 range), denominator via an
appended ones-column on V (M=65 matmuls), mask applied as an additive
per-partition bias (-1e18) inside the Exp activation.
"""

import os

import numpy as np
import ml_dtypes

B, S, D, H = 4, 1024, 1024, 16
DH = D // H
SCALE = DH ** -0.5
NEG = -1e18
P = 128
HLOC = H // 2          # heads per core
DHALF = D // 2         # projection cols per core
ET = D // P            # 8 contraction tiles over D
DC = DHALF // P        # 4 chunks of local head dims
KC = S // P            # 8 key chunks
NW = 512               # matmul moving width
NQ2 = S // NW          # 2 query halves
bf16 = ml_dtypes.bfloat16

_CACHE: dict = {}


def _emit(tc, nc, aps):
    import concourse.bass as bass
    from concourse import mybir

    f32 = mybir.dt.float32
    bf = mybir.dt.bfloat16
    xq, xk, xv, wq, wk, wv, wo, mb, outp, attwT = aps

    from contextlib import ExitStack

    es = ExitStack()
    consts = es.enter_context(tc.tile_pool(name="consts", bufs=1))
    xpool = es.enter_context(tc.tile_pool(name="xpool", bufs=1))
    wpool = es.enter_context(tc.tile_pool(name="wpool", bufs=1))
    wopool = es.enter_context(tc.tile_pool(name="wopool", bufs=1))
    persist = es.enter_context(tc.tile_pool(name="persist", bufs=1))
    epool = es.enter_context(tc.tile_pool(name="epool", bufs=1))
    stage = es.enter_context(tc.tile_pool(name="stage", bufs=2))
    small = es.enter_context(tc.tile_pool(name="small", bufs=1))
    # PSUM budget (8 banks): l0/l1 [128,1024] = 4 banks, c0/c1 [128,1024] = 4.
    psL = es.enter_context(tc.tile_pool(name="psL", bufs=1, space="PSUM"))
    psC = es.enter_context(tc.tile_pool(name="psC", bufs=1, space="PSUM"))

    _lc = [0]
    _ln = [0]

    def ltile(cols):
        _lc[0] ^= 1
        _ln[0] += 1
        return psL.tile([P, cols], f32, tag=f"l{_lc[0]}", name=f"lt{_ln[0]}")

    with es:
        mb_sb = consts.tile([P, KC], f32, tag="mb")
        nc.sync.dma_start(out=mb_sb[:], in_=mb[:])
        mbh_sb = consts.tile([P, KC], f32, tag="mbh")
        nc.vector.tensor_scalar_mul(out=mbh_sb[:], in0=mb_sb[:], scalar1=0.5)

        # ---- inputs (bf16, feature-major), split per contraction tile so the
        # first projection matmuls start as soon as the first slices land.
        def load_split(pool, dram, name, width):
            dv = dram.rearrange("(t p) m -> t p m", p=P)
            tiles = []
            for t in range(ET):
                tl = pool.tile([P, width], bf, tag=f"{name}{t}", name=f"{name}{t}")
                nc.sync.dma_start(out=tl[:], in_=dv[t])
                tiles.append(tl)
            return tiles

        wq_sb = load_split(wpool, wq, "wq", DHALF)
        xq_sb = load_split(xpool, xq, "xq", S)
        wk_sb = load_split(wpool, wk, "wk", DHALF)
        xk_sb = load_split(xpool, xk, "xk", S)

        QTc = [persist.tile([P, S], bf, tag=f"QT{i}", name=f"QT{i}") for i in range(DC)]
        KTc = [persist.tile([P, S], bf, tag=f"KT{i}", name=f"KT{i}") for i in range(DC)]
        Vc = [
            persist.tile([P, HLOC * 65], bf, tag=f"V{k}", name=f"V{k}")
            for k in range(KC)
        ]
        ctxc = [
            persist.tile([P, S], bf, tag=f"CX{i}", name=f"CX{i}") for i in range(DC)
        ]

        def proj_chunk(dst, w_sb, x_sb, i):
            # one 128-col chunk of a q/k projection, as two half-groups so the
            # PSUM slot is held only ~2us at a time
            for n in range(NQ2):
                ps = ltile(NW)
                for t in range(ET):
                    nc.tensor.matmul(
                        ps[:],
                        lhsT=w_sb[t][:, i * P:(i + 1) * P],
                        rhs=x_sb[t][:, n * NW:(n + 1) * NW],
                        start=(t == 0),
                        stop=(t == ET - 1),
                    )
                nc.vector.tensor_copy(out=dst[:, n * NW:(n + 1) * NW], in_=ps[:])

        proj_chunk(QTc[0], wq_sb, xq_sb, 0)
        proj_chunk(KTc[0], wk_sb, xk_sb, 0)

        # ---- V projection inputs (the projection itself is interleaved into
        # pair 0's chunk loop below: it is DMA-gated early on)
        wv_sb = load_split(wpool, wv, "wv", DHALF)
        xv_sb = load_split(xpool, xv, "xv", S)
        wo_sb = wopool.tile([P, DC, D], bf, tag="wo")
        nc.sync.dma_start(out=wo_sb[:], in_=wo.rearrange("(i p) n -> p i n", p=P))

        # ---- head pairs: logits -> exp -> ctx.  PE filler is interleaved
        # INSIDE each pair's chunk loop (the per-engine instruction order is
        # static, so filler between pairs would block the next pair's ACT
        # feed): pairs 0-2 carry the next q/k projection chunk, pair 3
        # carries the attention-weight head-sum groups.
        def attw_block(ck):
            # attwT_partial[ck] = (KT^T QT)/H + mask/2, one 128-row block
            st = stage.tile([P, S], f32, tag="attw_st", name=f"awst{ck}")
            for n in range(NQ2):
                ps = ltile(NW)
                for i in range(DC):
                    nc.tensor.matmul(
                        ps[:],
                        lhsT=KTc[i][:, ck * P:(ck + 1) * P],
                        rhs=QTc[i][:, n * NW:(n + 1) * NW],
                        start=(i == 0),
                        stop=(i == DC - 1),
                    )
                nc.vector.tensor_scalar(
                    out=st[:, n * NW:(n + 1) * NW],
                    in0=ps[:],
                    scalar1=1.0 / H,
                    scalar2=mbh_sb[:, ck:ck + 1],
                    op0=mybir.AluOpType.mult,
                    op1=mybir.AluOpType.add,
                )
            nc.sync.dma_start(out=attwT[ck * P:(ck + 1) * P, :], in_=st[:])

        def proj_half(dst, w_sb, x_sb, i, n):
            ps = ltile(NW)
            for t in range(ET):
                nc.tensor.matmul(
                    ps[:],
                    lhsT=w_sb[t][:, i * P:(i + 1) * P],
                    rhs=x_sb[t][:, n * NW:(n + 1) * NW],
                    start=(t == 0),
                    stop=(t == ET - 1),
                )
            nc.vector.tensor_copy(out=dst[:, n * NW:(n + 1) * NW], in_=ps[:])

        for hp in range(DC):
            pscs = [
                psC.tile([P, S], f32, tag=f"c{s}", name=f"c{s}_{hp}")
                for s in range(2)
            ]
            e_all = {}

            def logits_exp(ck):
                e_tiles = [
                    epool.tile([P, S], bf, tag=f"e{s}_{ck}", name=f"e{s}_{ck}_{hp}")
                    for s in range(2)
                ]
                for s in range(2):
                    po = 64 * s
                    psl = ltile(S)
                    for n in range(NQ2):
                        nc.tensor.matmul(
                            psl[:, n * NW:(n + 1) * NW],
                            lhsT=KTc[hp][po:po + 64, ck * P:(ck + 1) * P],
                            rhs=QTc[hp][po:po + 64, n * NW:(n + 1) * NW],
                            start=True,
                            stop=True,
                        )
                    nc.scalar.activation(
                        out=e_tiles[s][:],
                        in_=psl[:],
                        func=mybir.ActivationFunctionType.Exp,
                        bias=mb_sb[:, ck:ck + 1],
                        scale=1.0,
                    )
                e_all[ck] = e_tiles

            def ctx_mms(ck):
                for s in range(2):
                    h = 2 * hp + s
                    for n in range(NQ2):
                        nc.tensor.matmul(
                            pscs[s][0:65, n * NW:(n + 1) * NW],
                            lhsT=Vc[ck][:, h * 65:(h + 1) * 65],
                            rhs=e_all[ck][s][:, n * NW:(n + 1) * NW],
                            start=(ck == 0),
                            stop=(ck == KC - 1),
                        )

            if hp == 0:
                # V projection is DMA-gated early on; run all of pair 0's
                # logits/exp first so ACT starts as soon as k-proj chunk 0
                # lands, then interleave the V-projection with ctx matmuls.
                for ck in range(KC):
                    logits_exp(ck)
                for ck in range(KC):
                    ps = ltile(NW)
                    for t in range(ET):
                        nc.tensor.matmul(
                            ps[:],
                            lhsT=xv_sb[t][:, ck * P:(ck + 1) * P],
                            rhs=wv_sb[t][:],
                            start=(t == 0),
                            stop=(t == ET - 1),
                        )
                    dst = Vc[ck][:].rearrange("p (h c) -> p h c", c=65)
                    nc.vector.tensor_copy(
                        out=dst[:, :, 0:64],
                        in_=ps[:].rearrange("p (h c) -> p h c", c=64),
                    )
                    nc.vector.memset(dst[:, :, 64:65], 1.0)
                    ctx_mms(ck)
            else:
                for ck in range(KC):
                    logits_exp(ck)
                    ctx_mms(ck)
                    if hp < DC - 1:
                        if ck % 2 == 1:
                            j = (ck - 1) // 2
                            w_sb, x_sb, dst = (
                                (wq_sb, xq_sb, QTc[hp + 1])
                                if j < 2
                                else (wk_sb, xk_sb, KTc[hp + 1])
                            )
                            proj_half(dst, w_sb, x_sb, hp + 1, j % 2)
                    else:
                        attw_block(ck)
            if hp == 0:
                # pair 1's projection chunk (pair 0's loop was full with V)
                proj_half(QTc[1], wq_sb, xq_sb, 1, 0)
                proj_half(QTc[1], wq_sb, xq_sb, 1, 1)
                proj_half(KTc[1], wk_sb, xk_sb, 1, 0)
                proj_half(KTc[1], wk_sb, xk_sb, 1, 1)
            # Drain ctx PSUM to SBUF promptly (releases the PSUM slot for the
            # next pair); normalization then runs on DVE/GPSIMD off the
            # critical path.
            den = small.tile([1, 2 * S], f32, tag="den", name=f"den{hp}")
            cus = []
            for s in range(2):
                cu = small.tile([64, S], bf, tag=f"cu{s}", name=f"cu{s}_{hp}")
                nc.vector.tensor_copy(out=cu[:], in_=pscs[s][0:64, :])
                nc.vector.tensor_copy(
                    out=den[0:1, s * S:(s + 1) * S], in_=pscs[s][64:65, :]
                )
                cus.append(cu)
            nc.vector.reciprocal_approx_fast(out=den[:], in_=den[:])
            for s in range(2):
                rb = small.tile([64, S], f32, tag=f"rb{s}", name=f"rb{s}_{hp}")
                nc.gpsimd.partition_broadcast(rb[:], den[0:1, s * S:(s + 1) * S])
                nc.gpsimd.tensor_mul(
                    out=ctxc[hp][64 * s:64 * s + 64, :],
                    in0=cus[s][:],
                    in1=rb[:],
                )

        # ---- output projection: outp_partial = ctxT^T @ Wo_rows
        for m in range(KC):
            ps = ltile(S)
            for n in range(NQ2):
                for i in range(DC):
                    nc.tensor.matmul(
                        ps[:, n * NW:(n + 1) * NW],
                        lhsT=ctxc[i][:, m * P:(m + 1) * P],
                        rhs=wo_sb[:, i, n * NW:(n + 1) * NW],
                        start=(i == 0),
                        stop=(i == DC - 1),
                    )
            st = stage.tile([P, S], f32, tag="out_st", name=f"ost{m}")
            nc.vector.tensor_copy(out=st[:], in_=ps[:])
            nc.sync.dma_start(out=outp[m * P:(m + 1) * P, :], in_=st[:])


def build():
    if "nc" in _CACHE:
        return _CACHE["nc"]
    import concourse.tile as tile
    from concourse import bacc, mybir

    f32 = mybir.dt.float32
    bf = mybir.dt.bfloat16
    nc = bacc.Bacc("TRN2", target_bir_lowering=False, debug=False)
    xq = nc.dram_tensor("xq", [D, S], bf, kind="ExternalInput").ap()
    xk = nc.dram_tensor("xk", [D, S], bf, kind="ExternalInput").ap()
    xv = nc.dram_tensor("xv", [D, S], bf, kind="ExternalInput").ap()
    wq = nc.dram_tensor("wq", [D, DHALF], bf, kind="ExternalInput").ap()
    wk = nc.dram_tensor("wk", [D, DHALF], bf, kind="ExternalInput").ap()
    wv = nc.dram_tensor("wv", [D, DHALF], bf, kind="ExternalInput").ap()
    wo = nc.dram_tensor("wo", [DHALF, D], bf, kind="ExternalInput").ap()
    mb = nc.dram_tensor("mb", [P, KC], f32, kind="ExternalInput").ap()
    outp = nc.dram_tensor("outp", [S, D], f32, kind="ExternalOutput").ap()
    attwT = nc.dram_tensor("attwT", [S, S], f32, kind="ExternalOutput").ap()

    with tile.TileContext(nc) as tc:
        _emit(tc, nc, (xq, xk, xv, wq, wk, wv, wo, mb, outp, attwT))
    nc.compile()
    _CACHE["nc"] = nc
    return nc


def make_in_maps(queries, keys, values, Wq, Wk, Wv, Wo, mask):
    queries = np.asarray(queries, np.float32)
    keys = np.asarray(keys, np.float32)
    values = np.asarray(values, np.float32)
    Wqs = np.asarray(Wq, np.float32) * SCALE
    Wk = np.asarray(Wk, np.float32)
    Wv = np.asarray(Wv, np.float32)
    Wo = np.asarray(Wo, np.float32)
    mask = np.asarray(mask)

    xT = {}
    for b in range(B):
        xT[b] = tuple(
            np.ascontiguousarray(a[b].T).astype(bf16)
            for a in (queries, keys, values)
        )
    in_maps = []
    for c in range(8):
        b, g = divmod(c, 2)
        sl = slice(g * DHALF, (g + 1) * DHALF)
        mbias = np.where(mask[b, 0], np.float32(NEG), np.float32(0.0)).astype(
            np.float32
        )
        in_maps.append(
            {
                "xq": xT[b][0],
                "xk": xT[b][1],
                "xv": xT[b][2],
                "wq": np.ascontiguousarray(Wqs[:, sl]).astype(bf16),
                "wk": np.ascontiguousarray(Wk[:, sl]).astype(bf16),
                "wv": np.ascontiguousarray(Wv[:, sl]).astype(bf16),
                "wo": np.ascontiguousarray(Wo[sl, :]).astype(bf16),
                "mb": np.ascontiguousarray(mbias.reshape(KC, P).T),
            }
        )
    return in_maps


def gather(results):
    outputs = np.empty((B, S, D), np.float32)
    attw = np.empty((B, S, S), np.float32)
    for b in range(B):
        outputs[b] = results[2 * b]["outp"] + results[2 * b + 1]["outp"]
        attw[b] = (results[2 * b]["attwT"] + results[2 * b + 1]["attwT"]).T
    return outputs, attw


def _enable_ntff_tracing():
    """Bridge trn_agent_boot's ctypes NTFF hook into antenv.axon_hooks
    (absent on this image), so run_bass_kernel_spmd(trace=True) can pull
    NTFF profiles back from the terminal. Also neuter the S3 artifact
    upload, which has no credentials here."""
    import sys
    import types

    try:
        from antenv.axon_hooks import get_axon_ntff_profile_hook  # noqa: F401
    except ImportError:
        import antenv
        from trn_agent_boot.trn_boot import _ntff_profile_via_ctypes

        hook = _ntff_profile_via_ctypes("/opt/axon/libaxon_pjrt.so")
        mod = types.ModuleType("antenv.axon_hooks")
        mod.get_axon_ntff_profile_hook = lambda: hook
        mod.set_axon_ntff_profile_hook = lambda h: None
        sys.modules["antenv.axon_hooks"] = mod
        antenv.axon_hooks = mod

    import concourse.bass_utils as bu

    bu.upload_artifacts = lambda tmpdir: tmpdir


def run(inputs, trace=False):
    from concourse.bass_utils import run_bass_kernel_spmd

    if trace:
        _enable_ntff_tracing()
    nc = build()
    in_maps = make_in_maps(**inputs)
    res = run_bass_kernel_spmd(nc, in_maps, list(range(8)), trace=trace)
    outputs, attw = gather(res.results)
    return (outputs, attw), res


def kernel(queries, keys, values, Wq, Wk, Wv, Wo, mask):
    trace = bool(os.environ.get("KERNEL_TRACE"))
    (outputs, attw), res = run(
        dict(
            queries=queries,
            keys=keys,
            values=values,
            Wq=Wq,
            Wk=Wk,
            Wv=Wv,
            Wo=Wo,
            mask=mask,
        ),
        trace=trace,
    )
    if trace and res.exec_time_ns is not None:
        print(f"HW exec time: {res.exec_time_ns} ns")
    return outputs, attw


# revision 20
# speedup vs baseline: 1.5623x; 1.1749x over previous
"""MultiHeadAttention Trainium2 Bass kernel (8 NeuronCores, SPMD, no collectives).

Problem (hardcoded): B=4, S=1024, D=1024, H=16, DH=64, fp32 I/O.
  q = split_heads(queries @ Wq) * DH**-0.5
  k = split_heads(keys @ Wk); v = split_heads(values @ Wv)
  logits = q k^T, masked (key mask -> -1e18)
  attention_weights = logits.sum(heads)/H      [B,S,S]
  outputs = (softmax(logits) v) @ Wo           [B,S,D]

Sharding: core c -> (batch b = c//2, head-half g = c%2).  Each core computes
8 heads' worth of projections/attention for one batch and a PARTIAL
  - outputs contribution  (its heads' rows of Wo)        [S,D] f32
  - attention-weight head-sum, stored transposed [ks,qs] [S,S] f32
Host sums the two partials per batch (and transposes attw).  No device
collectives; host-side shard/unshard is numpy.

Device layouts are feature-major ("transposed"): host passes queries^T etc.
All matmul inputs bf16, PSUM accumulation f32.  Softmax without max-
subtraction (logits are O(10); exp stays in fp32 range), denominator via an
appended ones-column on V (M=65 matmuls), mask applied as an additive
per-partition bias (-1e18) inside the Exp activation.
"""

import os

import numpy as np
import ml_dtypes

B, S, D, H = 4, 1024, 1024, 16
DH = D // H
SCALE = DH ** -0.5
NEG = -1e18
P = 128
HLOC = H // 2          # heads per core
DHALF = D // 2         # projection cols per core
ET = D // P            # 8 contraction tiles over D
DC = DHALF // P        # 4 chunks of local head dims
KC = S // P            # 8 key chunks
NW = 512               # matmul moving width
NQ2 = S // NW          # 2 query halves
bf16 = ml_dtypes.bfloat16

_CACHE: dict = {}


def _emit(tc, nc, aps):
    import concourse.bass as bass
    from concourse import mybir

    f32 = mybir.dt.float32
    bf = mybir.dt.bfloat16
    xq, xk, xv, wq, wk, wv, wo, mb, outp, attwT = aps

    from contextlib import ExitStack

    es = ExitStack()
    consts = es.enter_context(tc.tile_pool(name="consts", bufs=1))
    xpool = es.enter_context(tc.tile_pool(name="xpool", bufs=2))
    wpool = es.enter_context(tc.tile_pool(name="wpool", bufs=2))
    wopool = es.enter_context(tc.tile_pool(name="wopool", bufs=1))
    persist = es.enter_context(tc.tile_pool(name="persist", bufs=1))
    epool = es.enter_context(tc.tile_pool(name="epool", bufs=2))
    stage = es.enter_context(tc.tile_pool(name="stage", bufs=2))
    small = es.enter_context(tc.tile_pool(name="small", bufs=1))
    # PSUM budget (8 banks): l0/l1 [128,1024] = 4 banks, c0/c1 [128,1024] = 4.
    psL = es.enter_context(tc.tile_pool(name="psL", bufs=1, space="PSUM"))
    psC = es.enter_context(tc.tile_pool(name="psC", bufs=1, space="PSUM"))

    _lc = [0]

    def ltile():
        _lc[0] ^= 1
        return psL.tile([P, S], f32, tag=f"l{_lc[0]}", name=f"lt{_lc[0]}")

    with es:
        mb_sb = consts.tile([P, KC], f32, tag="mb")
        nc.sync.dma_start(out=mb_sb[:], in_=mb[:])
        mbh_sb = consts.tile([P, KC], f32, tag="mbh")
        nc.vector.tensor_scalar_mul(out=mbh_sb[:], in0=mb_sb[:], scalar1=0.5)

        # ---- projections: QT/KT [P, DC, S] (head-dim-major), V_aug [P, KC, HLOC*65]
        QT = persist.tile([P, DC, S], bf, tag="QT")
        KT = persist.tile([P, DC, S], bf, tag="KT")
        Vaug = persist.tile([P, KC, HLOC * 65], bf, tag="Vaug")

        for name, x_d, w_d, dstT in (("q", xq, wq, QT), ("k", xk, wk, KT)):
            x_sb = xpool.tile([P, ET, S], bf, tag="x", name=f"x{name}")
            nc.sync.dma_start(out=x_sb[:], in_=x_d.rearrange("(t p) s -> p t s", p=P))
            w_sb = wpool.tile([P, ET, DHALF], bf, tag="w", name=f"w{name}")
            nc.sync.dma_start(out=w_sb[:], in_=w_d.rearrange("(t p) m -> p t m", p=P))
            for i in range(DC):
                ps = ltile()
                for n in range(NQ2):
                    for t in range(ET):
                        nc.tensor.matmul(
                            ps[:, n * NW:(n + 1) * NW],
                            lhsT=w_sb[:, t, i * P:(i + 1) * P],
                            rhs=x_sb[:, t, n * NW:(n + 1) * NW],
                            start=(t == 0),
                            stop=(t == ET - 1),
                        )
                nc.vector.tensor_copy(out=dstT[:, i, :], in_=ps[:])

        xv_sb = xpool.tile([P, ET, S], bf, tag="x", name="xv")
        nc.sync.dma_start(out=xv_sb[:], in_=xv.rearrange("(t p) s -> p t s", p=P))
        wv_sb = wpool.tile([P, ET, DHALF], bf, tag="w", name="wv")
        nc.sync.dma_start(out=wv_sb[:], in_=wv.rearrange("(t p) m -> p t m", p=P))
        wo_sb = wopool.tile([P, DC, D], bf, tag="wo")
        nc.sync.dma_start(out=wo_sb[:], in_=wo.rearrange("(i p) n -> p i n", p=P))
        for ck in range(0, KC, 2):
            ps = ltile()
            for half in range(2):
                for t in range(ET):
                    nc.tensor.matmul(
                        ps[:, half * NW:(half + 1) * NW],
                        lhsT=xv_sb[:, t, (ck + half) * P:(ck + half + 1) * P],
                        rhs=wv_sb[:, t, :],
                        start=(t == 0),
                        stop=(t == ET - 1),
                    )
            for half in range(2):
                dst = Vaug[:, ck + half, :].rearrange("p (h c) -> p h c", c=65)
                nc.vector.tensor_copy(
                    out=dst[:, :, 0:64],
                    in_=ps[:, half * NW:(half + 1) * NW].rearrange(
                        "p (h c) -> p h c", c=64
                    ),
                )
                nc.vector.memset(dst[:, :, 64:65], 1.0)

        # ---- per head-pair: logits -> exp -> ctx (ones-column denominator).
        # Fine-grained per-(head,chunk) exp tiles keep PE/ACT pipelined.
        ctxT = persist.tile([P, DC, S], bf, tag="ctxT")
        for hp in range(DC):
            pscs = [
                psC.tile([P, S], f32, tag=f"c{s}", name=f"c{s}_{hp}")
                for s in range(2)
            ]
            for ck in range(KC):
                e_tiles = [
                    epool.tile([P, S], bf, tag=f"e{s}_{ck}", name=f"e{s}_{ck}_{hp}")
                    for s in range(2)
                ]
                for s in range(2):
                    po = 64 * s
                    psl = ltile()
                    for n in range(NQ2):
                        nc.tensor.matmul(
                            psl[:, n * NW:(n + 1) * NW],
                            lhsT=KT[po:po + 64, hp, ck * P:(ck + 1) * P],
                            rhs=QT[po:po + 64, hp, n * NW:(n + 1) * NW],
                            start=True,
                            stop=True,
                        )
                    nc.scalar.activation(
                        out=e_tiles[s][:],
                        in_=psl[:],
                        func=mybir.ActivationFunctionType.Exp,
                        bias=mb_sb[:, ck:ck + 1],
                        scale=1.0,
                    )
                for s in range(2):
                    h = 2 * hp + s
                    for n in range(NQ2):
                        nc.tensor.matmul(
                            pscs[s][0:65, n * NW:(n + 1) * NW],
                            lhsT=Vaug[:, ck, h * 65:(h + 1) * 65],
                            rhs=e_tiles[s][:, n * NW:(n + 1) * NW],
                            start=(ck == 0),
                            stop=(ck == KC - 1),
                        )
            # Drain ctx PSUM to SBUF promptly (releases the PSUM slot for the
            # next pair); normalization then runs off the critical path.
            den = small.tile([1, 2 * S], f32, tag="den", name=f"den{hp}")
            cus = []
            for s in range(2):
                cu = small.tile([64, S], bf, tag=f"cu{s}", name=f"cu{s}_{hp}")
                nc.vector.tensor_copy(out=cu[:], in_=pscs[s][0:64, :])
                nc.vector.tensor_copy(
                    out=den[0:1, s * S:(s + 1) * S], in_=pscs[s][64:65, :]
                )
                cus.append(cu)
            nc.vector.reciprocal_approx_fast(out=den[:], in_=den[:])
            for s in range(2):
                rb = small.tile([64, S], f32, tag=f"rb{s}", name=f"rb{s}_{hp}")
                nc.gpsimd.partition_broadcast(rb[:], den[0:1, s * S:(s + 1) * S])
                nc.vector.tensor_mul(
                    out=ctxT[64 * s:64 * s + 64, hp, :],
                    in0=cus[s][:],
                    in1=rb[:],
                )

        # ---- attention-weight head-sum: attwT_partial = (KT^T QT)/H + mask/2
        for ck in range(KC):
            ps = psC.tile([P, S], f32, tag=f"c{ck % 2}", name=f"aw{ck}")
            for n in range(NQ2):
                for i in range(DC):
                    nc.tensor.matmul(
                        ps[:, n * NW:(n + 1) * NW],
                        lhsT=KT[:, i, ck * P:(ck + 1) * P],
                        rhs=QT[:, i, n * NW:(n + 1) * NW],
                        start=(i == 0),
                        stop=(i == DC - 1),
                    )
            st = stage.tile([P, S], f32, tag="attw_st", name=f"awst{ck}")
            nc.vector.tensor_scalar(
                out=st[:],
                in0=ps[:],
                scalar1=1.0 / H,
                scalar2=mbh_sb[:, ck:ck + 1],
                op0=mybir.AluOpType.mult,
                op1=mybir.AluOpType.add,
            )
            nc.sync.dma_start(out=attwT[ck * P:(ck + 1) * P, :], in_=st[:])

        # ---- output projection: outp_partial = ctxT^T @ Wo_rows
        for m in range(KC):
            ps = ltile()
            for n in range(NQ2):
                for i in range(DC):
                    nc.tensor.matmul(
                        ps[:, n * NW:(n + 1) * NW],
                        lhsT=ctxT[:, i, m * P:(m + 1) * P],
                        rhs=wo_sb[:, i, n * NW:(n + 1) * NW],
                        start=(i == 0),
                        stop=(i == DC - 1),
                    )
            st = stage.tile([P, S], f32, tag="out_st", name=f"ost{m}")
            nc.vector.tensor_copy(out=st[:], in_=ps[:])
            nc.sync.dma_start(out=outp[m * P:(m + 1) * P, :], in_=st[:])


def build():
    if "nc" in _CACHE:
        return _CACHE["nc"]
    import concourse.tile as tile
    from concourse import bacc, mybir

    f32 = mybir.dt.float32
    bf = mybir.dt.bfloat16
    nc = bacc.Bacc("TRN2", target_bir_lowering=False, debug=False)
    xq = nc.dram_tensor("xq", [D, S], bf, kind="ExternalInput").ap()
    xk = nc.dram_tensor("xk", [D, S], bf, kind="ExternalInput").ap()
    xv = nc.dram_tensor("xv", [D, S], bf, kind="ExternalInput").ap()
    wq = nc.dram_tensor("wq", [D, DHALF], bf, kind="ExternalInput").ap()
    wk = nc.dram_tensor("wk", [D, DHALF], bf, kind="ExternalInput").ap()
    wv = nc.dram_tensor("wv", [D, DHALF], bf, kind="ExternalInput").ap()
    wo = nc.dram_tensor("wo", [DHALF, D], bf, kind="ExternalInput").ap()
    mb = nc.dram_tensor("mb", [P, KC], f32, kind="ExternalInput").ap()
    outp = nc.dram_tensor("outp", [S, D], f32, kind="ExternalOutput").ap()
    attwT = nc.dram_tensor("attwT", [S, S], f32, kind="ExternalOutput").ap()

    with tile.TileContext(nc) as tc:
        _emit(tc, nc, (xq, xk, xv, wq, wk, wv, wo, mb, outp, attwT))
    nc.compile()
    _CACHE["nc"] = nc
    return nc


def make_in_maps(queries, keys, values, Wq, Wk, Wv, Wo, mask):
    queries = np.asarray(queries, np.float32)
    keys = np.asarray(keys, np.float32)
    values = np.asarray(values, np.float32)
    Wqs = np.asarray(Wq, np.float32) * SCALE
    Wk = np.asarray(Wk, np.float32)
    Wv = np.asarray(Wv, np.float32)
    Wo = np.asarray(Wo, np.float32)
    mask = np.asarray(mask)

    xT = {}
    for b in range(B):
        xT[b] = tuple(
            np.ascontiguousarray(a[b].T).astype(bf16)
            for a in (queries, keys, values)
        )
    in_maps = []
    for c in range(8):
        b, g = divmod(c, 2)
        sl = slice(g * DHALF, (g + 1) * DHALF)
        mbias = np.where(mask[b, 0], np.float32(NEG), np.float32(0.0)).astype(
            np.float32
        )
        in_maps.append(
            {
                "xq": xT[b][0],
                "xk": xT[b][1],
                "xv": xT[b][2],
                "wq": np.ascontiguousarray(Wqs[:, sl]).astype(bf16),
                "wk": np.ascontiguousarray(Wk[:, sl]).astype(bf16),
                "wv": np.ascontiguousarray(Wv[:, sl]).astype(bf16),
                "wo": np.ascontiguousarray(Wo[sl, :]).astype(bf16),
                "mb": np.ascontiguousarray(mbias.reshape(KC, P).T),
            }
        )
    return in_maps


def gather(results):
    outputs = np.empty((B, S, D), np.float32)
    attw = np.empty((B, S, S), np.float32)
    for b in range(B):
        outputs[b] = results[2 * b]["outp"] + results[2 * b + 1]["outp"]
        attw[b] = (results[2 * b]["attwT"] + results[2 * b + 1]["attwT"]).T
    return outputs, attw


def _enable_ntff_tracing():
    """Bridge trn_agent_boot's ctypes NTFF hook into antenv.axon_hooks
    (absent on this image), so run_bass_kernel_spmd(trace=True) can pull
    NTFF profiles back from the terminal. Also neuter the S3 artifact
    upload, which has no credentials here."""
    import sys
    import types

    try:
        from antenv.axon_hooks import get_axon_ntff_profile_hook  # noqa: F401
    except ImportError:
        import antenv
        from trn_agent_boot.trn_boot import _ntff_profile_via_ctypes

        hook = _ntff_profile_via_ctypes("/opt/axon/libaxon_pjrt.so")
        mod = types.ModuleType("antenv.axon_hooks")
        mod.get_axon_ntff_profile_hook = lambda: hook
        mod.set_axon_ntff_profile_hook = lambda h: None
        sys.modules["antenv.axon_hooks"] = mod
        antenv.axon_hooks = mod

    import concourse.bass_utils as bu

    bu.upload_artifacts = lambda tmpdir: tmpdir


def run(inputs, trace=False):
    from concourse.bass_utils import run_bass_kernel_spmd

    if trace:
        _enable_ntff_tracing()
    nc = build()
    in_maps = make_in_maps(**inputs)
    res = run_bass_kernel_spmd(nc, in_maps, list(range(8)), trace=trace)
    outputs, attw = gather(res.results)
    return (outputs, attw), res


def kernel(queries, keys, values, Wq, Wk, Wv, Wo, mask):
    trace = bool(os.environ.get("KERNEL_TRACE"))
    (outputs, attw), res = run(
        dict(
            queries=queries,
            keys=keys,
            values=values,
            Wq=Wq,
            Wk=Wk,
            Wv=Wv,
            Wo=Wo,
            mask=mask,
        ),
        trace=trace,
    )
    if trace and res.exec_time_ns is not None:
        print(f"HW exec time: {res.exec_time_ns} ns")
    return outputs, attw
